# revision 1
# baseline (speedup 1.0000x reference)
"""DFlashAttention Trainium2 kernel (8-core tensor-parallel over attention heads).

Shapes (hardcoded): D=2048, N=16 q-heads, K=8 kv-heads, H=128,
T_NOISE=2048 (query tokens), T_CTX=4096, S=6144 (kv tokens).

Sharding: core c owns q-heads {2c, 2c+1} and kv-head c (GQA groups=2).
Each core computes a partial (T, D) output (its 2 heads' slice of the
o-projection contraction); the host sums the 8 partials (TP unshard).

Layout strategy per core:
  - x_all^T [D, S] fed replicated (d on partitions = matmul contraction dim).
  - kv proj:  psum[s,0:128]=k, psum[s,128:256]=v  (one fp32r matmul chain,
    moving free dim 256).
  - RMSNorm over H via ACT Square+accum_out; RoPE via on-device sin/cos
    (angle mod 2pi + range wrap + ACT Sin); tables built once for all 48
    token tiles.
  - attention in [s, t] orientation: scores^T = kT.T @ qT (contraction H=128,
    single matmul per (s-tile, t-chunk)); exp on ACT (scale=1/sqrt(H) folded);
    no max subtraction (|score| <= sqrt(H)*1.1^2 ~ 13.7 after RMSNorm, exp is
    safe in fp32); row-sums via ones-matmul; A@V accumulates over s-tiles in
    PSUM with V in natural [s, h] layout.
  - softmax division deferred past the o-projection (denominator is constant
    along the contraction), where it is a per-partition scalar multiply.
"""

import sys

for _p in ("/opt/trn_rl_repo", "/root/.axon_site/_ro/trn_rl_repo"):
    if _p not in sys.path:
        sys.path.append(_p)

import math
import numpy as np

import concourse.bass as bass
import concourse.tile as tile
from concourse import bacc
from concourse import mybir
from concourse.bass_utils import run_bass_kernel_spmd
from concourse.masks import make_identity

D = 2048
N_HEADS = 16
K_HEADS = 8
H = 128
T_NOISE = 2048
T_CTX = 4096
S_ALL = T_CTX + T_NOISE          # 6144
EPS = 1e-6
ROPE_THETA = 1e6
N_CORES = 8
HEADS_PER_CORE = N_HEADS // N_CORES   # 2

P = 128                       # partition dim
S_TILES = S_ALL // P          # 48
T_TILES = T_NOISE // P        # 16
NOISE_TILE0 = T_CTX // P      # 32  (noise tokens are s-tiles 32..47)
D_TILES = D // P              # 16
FREE = 512                    # moving free-dim chunk
T_CHUNKS = T_NOISE // FREE    # 4
S_CHUNKS = S_ALL // FREE      # 12
D_CHUNKS = D // FREE          # 4

F32 = mybir.dt.float32
F32R = mybir.dt.float32r
MM_DT = F32R                  # dtype for all matmul operands

TWO_PI = 2.0 * math.pi
INV_SQRT_H = 1.0 / math.sqrt(H)

_CACHE = {}


def _build_program(reps=1):
    """Build the single-core SPMD bass program. Returns (nc, out_name).
    reps>1 repeats the whole kernel body (timing harness only)."""
    nc = bacc.Bacc("TRN2", target_bir_lowering=False, debug=False,
                   num_devices=N_CORES)

    xT = nc.dram_tensor("xT", [D, S_ALL], MM_DT, kind="ExternalInput").ap()
    wkv = nc.dram_tensor("wkv", [D, 2 * H], MM_DT, kind="ExternalInput").ap()
    wq = nc.dram_tensor("wq", [D, HEADS_PER_CORE * H], MM_DT,
                        kind="ExternalInput").ap()
    wo = nc.dram_tensor("wo", [HEADS_PER_CORE, H, D], MM_DT,
                        kind="ExternalInput").ap()
    posr = nc.dram_tensor("posr", [S_TILES, P, 1], F32,
                          kind="ExternalInput").ap()
    invfb = nc.dram_tensor("invfb", [P, H // 2], F32,
                           kind="ExternalInput").ap()
    qscaleb = nc.dram_tensor("qscaleb", [P, H], F32,
                             kind="ExternalInput").ap()
    kscaleb = nc.dram_tensor("kscaleb", [P, H], F32,
                             kind="ExternalInput").ap()
    onesb = nc.dram_tensor("onesb", [P, 1], MM_DT, kind="ExternalInput").ap()
    out = nc.dram_tensor("out", [T_NOISE, D], F32, kind="ExternalOutput").ap()

    with tile.TileContext(nc) as tc:
        for rep in range(reps):
            _emit(nc, tc, xT, wkv, wq, wo, posr, invfb, qscaleb, kscaleb,
                  onesb, out, pfx=f"r{rep}_")
    nc.compile()
    return nc, "out"


def _emit(nc, tc, xT, wkv, wq, wo, posr, invfb, qscaleb, kscaleb, onesb, out, pfx=""):
    import contextlib
    ctx = contextlib.ExitStack()
    with ctx:
        const = ctx.enter_context(tc.tile_pool(name=pfx + "const", bufs=1))
        persist = ctx.enter_context(tc.tile_pool(name=pfx + "persist", bufs=1))

        # ---- constants ----
        ident = const.tile([P, P], F32, tag="ident")
        make_identity(nc, ident[:])
        ones = const.tile([P, 1], MM_DT, tag="ones")
        nc.sync.dma_start(ones[:], onesb[:])
        invf_sb = const.tile([P, H // 2], F32, tag="invf")
        nc.sync.dma_start(invf_sb[:], invfb[:])
        qsc_sb = const.tile([P, H], F32, tag="qsc")
        nc.sync.dma_start(qsc_sb[:], qscaleb[:])
        ksc_sb = const.tile([P, H], F32, tag="ksc")
        nc.sync.dma_start(ksc_sb[:], kscaleb[:])
        eps_col = const.tile([P, 1], F32, tag="eps")
        nc.vector.memset(eps_col[:], EPS)
        pos_sb = const.tile([P, S_TILES], F32, tag="pos")
        for si in range(S_TILES):
            nc.sync.dma_start(pos_sb[:, si:si + 1], posr[si])

        wkv_sb = [const.tile([P, 2 * H], MM_DT, tag=f"wkv{d}", name=f"wkv{d}")
                  for d in range(D_TILES)]
        wq_sb = [const.tile([P, HEADS_PER_CORE * H], MM_DT, tag=f"wq{d}", name=f"wqs{d}")
                 for d in range(D_TILES)]
        for d in range(D_TILES):
            nc.sync.dma_start(wkv_sb[d][:], wkv[d * P:(d + 1) * P, :])
            nc.sync.dma_start(wq_sb[d][:], wq[d * P:(d + 1) * P, :])
        wo_sb = [const.tile([P, D], MM_DT, tag=f"wo{h}", name=f"wos{h}")
                 for h in range(HEADS_PER_CORE)]
        for h in range(HEADS_PER_CORE):
            nc.sync.dma_start(wo_sb[h][:], wo[h])

        # ---- persistent activations ----
        half = H // 2
        sin_all = persist.tile([P, S_TILES * half], F32, tag="sin")
        cos_all = persist.tile([P, S_TILES * half], F32, tag="cos")
        kT_sb = persist.tile([P, S_ALL], MM_DT, tag="kT")
        v_sb = persist.tile([P, S_ALL], MM_DT, tag="v")       # [s-tile, h] blocks
        qT_sb = persist.tile([P, HEADS_PER_CORE * T_NOISE], MM_DT, tag="qT")
        oT_sb = persist.tile([P, HEADS_PER_CORE * T_NOISE], MM_DT, tag="oT")
        r_all = persist.tile([1, HEADS_PER_CORE * T_NOISE], F32, tag="r")
        rcol = persist.tile([P, HEADS_PER_CORE * T_TILES], F32, tag="rcol")

        # ---- RoPE sin/cos tables for all 48 token tiles ----
        # angle = pos * inv_freq; range-reduce mod 2pi via Cody-Waite
        # (k = int(angle/2pi); red = ((ang - k*c1) - k*c2) - k*c3).
        CW1, CW2, CW3 = 6.28125, 0.0019353071693331003, 1.0253131677018246e-11
        HGRP = S_TILES // 2
        HW_ = HGRP * half
        with tc.tile_pool(name=pfx + "ropebuild", bufs=1) as rp:
            for g in range(2):
                ang = rp.tile([P, HW_], F32, tag="ang", name="ang")
                kq = rp.tile([P, HW_], F32, tag="kq", name="kq")
                ki = rp.tile([P, HW_], mybir.dt.int32, tag="ki", name="ki")
                wrap = rp.tile([P, HW_], F32, tag="wrap", name="wrap")
                for j in range(HGRP):
                    si = g * HGRP + j
                    nc.vector.tensor_scalar_mul(
                        ang[:, j * half:(j + 1) * half], invf_sb[:, :],
                        pos_sb[:, si:si + 1])
                nc.vector.tensor_scalar_mul(kq[:], ang[:], 1.0 / TWO_PI)
                nc.vector.tensor_copy(ki[:], kq[:])
                nc.vector.tensor_copy(kq[:], ki[:])
                nc.vector.cody_waite_cascade(ang[:], ang[:], kq[:],
                                             CW1, CW2, CW3)
                dst = slice(g * HW_, (g + 1) * HW_)
                nc.vector.add_range_wrap(wrap[:], ang[:], 0.0, math.pi, TWO_PI)
                nc.scalar.activation(sin_all[:, dst], wrap[:],
                                     mybir.ActivationFunctionType.Sin)
                nc.vector.add_range_wrap(wrap[:], ang[:], math.pi / 2, math.pi,
                                         TWO_PI)
                nc.scalar.activation(cos_all[:, dst], wrap[:],
                                     mybir.ActivationFunctionType.Sin)

        def norm_rope_transpose(src_psum, scale_sb, si, dst_sb, work, psum_t):
            """src_psum [P(tok),H] fp32 -> rms-norm*scale -> rope -> transpose
            -> dst_sb [P(h), 128 tok]. si = token-tile index for positions."""
            sq = work.tile([P, H], F32, tag="sq")
            ssq = work.tile([P, 1], F32, tag="ssq")
            nc.scalar.activation(sq[:], src_psum, mybir.ActivationFunctionType.Square,
                                 accum_out=ssq[:])
            rms = work.tile([P, 1], F32, tag="rms")
            nc.scalar.activation(rms[:], ssq[:], mybir.ActivationFunctionType.Sqrt,
                                 bias=eps_col[:], scale=1.0 / H)
            rinv = work.tile([P, 1], F32, tag="rinv")
            nc.vector.reciprocal(rinv[:], rms[:])
            xn = work.tile([P, H], F32, tag="xn")
            nc.vector.scalar_tensor_tensor(
                xn[:], src_psum, rinv[:], scale_sb[:],
                mybir.AluOpType.mult, mybir.AluOpType.mult)
            # rope
            co = cos_all[:, si * half:(si + 1) * half]
            sn = sin_all[:, si * half:(si + 1) * half]
            x1 = xn[:, 0:half]
            x2 = xn[:, half:H]
            t1 = work.tile([P, half], F32, tag="t1")
            t2 = work.tile([P, half], F32, tag="t2")
            xr = work.tile([P, H], F32, tag="xr")
            nc.vector.tensor_mul(t1[:], x1, co)
            nc.vector.tensor_mul(t2[:], x2, sn)
            nc.vector.tensor_sub(xr[:, 0:half], t1[:], t2[:])
            nc.vector.tensor_mul(t1[:], x2, co)
            nc.vector.tensor_mul(t2[:], x1, sn)
            nc.vector.tensor_add(xr[:, half:H], t1[:], t2[:])
            # transpose -> dst
            pt = psum_t.tile([P, P], F32, tag="pt")
            nc.tensor.transpose(pt[:], xr[:], ident[:])
            nc.vector.tensor_copy(dst_sb, pt[:])

        # ---- Phase A: K/V projection, norm+rope K, build kT and v ----
        with tc.tile_pool(name=pfx + "pa_x", bufs=3) as xp, \
             tc.tile_pool(name=pfx + "pa_ps", bufs=1, space="PSUM") as pskv, \
             tc.tile_pool(name=pfx + "pa_pt", bufs=2, space="PSUM") as pst, \
             tc.tile_pool(name=pfx + "pa_w", bufs=2) as work:
            for sc in range(S_CHUNKS):
                xt = [None] * D_TILES
                ps = [pskv.tile([P, 2 * H], F32, tag=f"kv{j}", name=f"pskv{j}") for j in range(4)]
                for d in range(D_TILES):
                    xt[d] = xp.tile([P, FREE], MM_DT, tag="xstage", name="xstage")
                    nc.sync.dma_start(
                        xt[d][:], xT[d * P:(d + 1) * P,
                                     sc * FREE:(sc + 1) * FREE])
                    for j in range(4):
                        nc.tensor.matmul(
                            ps[j][:], xt[d][:, j * P:(j + 1) * P],
                            wkv_sb[d][:], start=(d == 0), stop=(d == D_TILES - 1))
                for j in range(4):
                    si = sc * 4 + j
                    nc.vector.tensor_copy(v_sb[:, si * P:(si + 1) * P],
                                          ps[j][:, H:2 * H])
                    norm_rope_transpose(ps[j][:, 0:H], ksc_sb, si,
                                        kT_sb[:, si * P:(si + 1) * P],
                                        work, pst)

        # ---- Phase B: Q projection, norm+rope, build qT (2 heads) ----
        with tc.tile_pool(name=pfx + "pb_x", bufs=3) as xp, \
             tc.tile_pool(name=pfx + "pb_ps", bufs=1, space="PSUM") as psq, \
             tc.tile_pool(name=pfx + "pb_pt", bufs=2, space="PSUM") as pst, \
             tc.tile_pool(name=pfx + "pb_w", bufs=2) as work:
            for tch in range(T_CHUNKS):
                xt = [None] * D_TILES
                ps = [psq.tile([P, HEADS_PER_CORE * H], F32, tag=f"q{j}", name=f"psq{j}")
                      for j in range(4)]
                for d in range(D_TILES):
                    xt[d] = xp.tile([P, FREE], MM_DT, tag="xstage", name="xstage")
                    nc.sync.dma_start(
                        xt[d][:], xT[d * P:(d + 1) * P,
                                     T_CTX + tch * FREE:T_CTX + (tch + 1) * FREE])
                    for j in range(4):
                        nc.tensor.matmul(
                            ps[j][:], xt[d][:, j * P:(j + 1) * P],
                            wq_sb[d][:], start=(d == 0), stop=(d == D_TILES - 1))
                for j in range(4):
                    ti = tch * 4 + j
                    for hh in range(HEADS_PER_CORE):
                        norm_rope_transpose(
                            ps[j][:, hh * H:(hh + 1) * H], qsc_sb,
                            NOISE_TILE0 + ti,
                            qT_sb[:, hh * T_NOISE + ti * P:
                                  hh * T_NOISE + (ti + 1) * P],
                            work, pst)

        # ---- Phase C: attention ----
        PAIR = 2 * FREE   # exp processes two score banks at once
        with tc.tile_pool(name=pfx + "pc_sc", bufs=2, space="PSUM") as psc, \
             tc.tile_pool(name=pfx + "pc_av", bufs=2, space="PSUM") as pav, \
             tc.tile_pool(name=pfx + "pc_r", bufs=2, space="PSUM") as pr, \
             tc.tile_pool(name=pfx + "pc_exp", bufs=3) as pexp:
            for hh in range(HEADS_PER_CORE):
                for tch in range(T_CHUNKS):
                    qslice = qT_sb[:, hh * T_NOISE + tch * FREE:
                                   hh * T_NOISE + (tch + 1) * FREE]
                    av = pav.tile([P, FREE], F32, tag="av")
                    rr = pr.tile([1, FREE], F32, tag="rr")
                    for sp in range(S_TILES // 2):
                        sc_ps = psc.tile([P, PAIR], F32, tag="sc")
                        ex = pexp.tile([P, PAIR], MM_DT, tag="ex")
                        for u in range(2):
                            si = sp * 2 + u
                            nc.tensor.matmul(
                                sc_ps[:, u * FREE:(u + 1) * FREE],
                                kT_sb[:, si * P:(si + 1) * P], qslice,
                                start=True, stop=True)
                        nc.scalar.activation(ex[:], sc_ps[:],
                                             mybir.ActivationFunctionType.Exp,
                                             scale=INV_SQRT_H)
                        for u in range(2):
                            si = sp * 2 + u
                            first = si == 0
                            last = si == S_TILES - 1
                            nc.tensor.matmul(
                                av[:], v_sb[:, si * P:(si + 1) * P],
                                ex[:, u * FREE:(u + 1) * FREE],
                                start=first, stop=last)
                            nc.tensor.matmul(
                                rr[:], ones[:],
                                ex[:, u * FREE:(u + 1) * FREE],
                                start=first, stop=last)
                    nc.vector.tensor_copy(
                        oT_sb[:, hh * T_NOISE + tch * FREE:
                              hh * T_NOISE + (tch + 1) * FREE], av[:])
                    nc.vector.reciprocal(
                        r_all[0:1, hh * T_NOISE + tch * FREE:
                              hh * T_NOISE + (tch + 1) * FREE], rr[:])

        # recip row -> per-partition columns (SBUF->SBUF DMA transpose, tiny)
        for hh in range(HEADS_PER_CORE):
            for ti in range(T_TILES):
                nc.sync.dma_start(
                    rcol[:, hh * T_TILES + ti:hh * T_TILES + ti + 1],
                    r_all[0:1, hh * T_NOISE + ti * P:hh * T_NOISE + (ti + 1) * P])

        # ---- Phase D: o-projection + deferred softmax normalization ----
        with tc.tile_pool(name=pfx + "pd_ps", bufs=2, space="PSUM") as pso, \
             tc.tile_pool(name=pfx + "pd_w", bufs=3) as work:
            for ti in range(T_TILES):
                for dc in range(D_CHUNKS):
                    po = [pso.tile([P, FREE], F32, tag=f"po{h}", name=f"po{h}")
                          for h in range(HEADS_PER_CORE)]
                    for h in range(HEADS_PER_CORE):
                        nc.tensor.matmul(
                            po[h][:],
                            oT_sb[:, h * T_NOISE + ti * P:h * T_NOISE + (ti + 1) * P],
                            wo_sb[h][:, dc * FREE:(dc + 1) * FREE],
                            start=True, stop=True)
                    tmp = work.tile([P, FREE], F32, tag="tmp")
                    nc.vector.tensor_scalar_mul(
                        tmp[:], po[1][:],
                        rcol[:, T_TILES + ti:T_TILES + ti + 1])
                    ot = work.tile([P, FREE], F32, tag="ot")
                    nc.vector.scalar_tensor_tensor(
                        ot[:], po[0][:], rcol[:, ti:ti + 1], tmp[:],
                        mybir.AluOpType.mult, mybir.AluOpType.add)
                    nc.sync.dma_start(
                        out[ti * P:(ti + 1) * P, dc * FREE:(dc + 1) * FREE],
                        ot[:])


def _get_program(reps=1):
    key = f"prog{reps}"
    if key not in _CACHE:
        _CACHE[key] = _build_program(reps)
    return _CACHE[key]


def prepare_in_maps(x_noise, target_hidden, Wq, Wk, Wv, Wo, q_scale, k_scale,
                    noise_positions, ctx_positions):
    x_noise = np.asarray(x_noise, dtype=np.float32)
    target_hidden = np.asarray(target_hidden, dtype=np.float32)
    Wq = np.asarray(Wq, dtype=np.float32)
    Wk = np.asarray(Wk, dtype=np.float32)
    Wv = np.asarray(Wv, dtype=np.float32)
    Wo = np.asarray(Wo, dtype=np.float32)
    q_scale = np.asarray(q_scale, dtype=np.float32)
    k_scale = np.asarray(k_scale, dtype=np.float32)

    x_all = np.concatenate([target_hidden, x_noise], axis=0)       # (S, D)
    xT = np.ascontiguousarray(x_all.T)                             # (D, S)
    pos_all = np.concatenate(
        [np.asarray(ctx_positions), np.asarray(noise_positions)]
    ).astype(np.float32)
    posr = np.ascontiguousarray(pos_all.reshape(S_TILES, P, 1))
    half = H // 2
    inv_freq = (ROPE_THETA ** (-np.arange(half, dtype=np.float32) * 2.0 / H)
                ).astype(np.float32)
    invfb = np.ascontiguousarray(np.broadcast_to(inv_freq, (P, half)))
    qscaleb = np.ascontiguousarray(np.broadcast_to(q_scale, (P, H)))
    kscaleb = np.ascontiguousarray(np.broadcast_to(k_scale, (P, H)))

    in_maps = []
    for c in range(N_CORES):
        wkv = np.ascontiguousarray(
            np.concatenate([Wk[:, c, :], Wv[:, c, :]], axis=1))     # (D, 256)
        wq = np.ascontiguousarray(
            Wq[:, c * HEADS_PER_CORE:(c + 1) * HEADS_PER_CORE, :]
            .reshape(D, HEADS_PER_CORE * H))                        # (D, 256)
        wo = np.ascontiguousarray(
            Wo[c * HEADS_PER_CORE:(c + 1) * HEADS_PER_CORE])        # (2,128,D)
        in_maps.append({
            "xT": xT, "wkv": wkv, "wq": wq, "wo": wo,
            "posr": posr, "invfb": invfb,
            "qscaleb": qscaleb, "kscaleb": kscaleb,
            "onesb": np.ones((P, 1), dtype=np.float32),
        })
    return in_maps


def kernel(**inputs):
    in_maps = prepare_in_maps(**inputs)
    nc, out_name = _get_program()
    res = run_bass_kernel_spmd(nc, in_maps, core_ids=list(range(N_CORES)))
    acc = np.zeros((T_NOISE, D), dtype=np.float32)
    for r in res.results:
        acc += r[out_name]
    return acc


def run_traced(inputs, **kw):
    """Run once with NTFF tracing; returns BassKernelResults (exec_time_ns)."""
    in_maps = prepare_in_maps(**inputs)
    nc, out_name = _get_program()
    return run_bass_kernel_spmd(nc, in_maps, core_ids=list(range(N_CORES)),
                                trace=True, **kw)



# revision 5
# speedup vs baseline: 1.4412x; 1.4412x over previous
"""DFlashAttention Trainium2 kernel (8-core tensor-parallel over attention heads).

Shapes (hardcoded): D=2048, N=16 q-heads, K=8 kv-heads, H=128,
T_NOISE=2048 (query tokens), T_CTX=4096, S=6144 (kv tokens).

Sharding: core c owns q-heads {2c, 2c+1} and kv-head c (GQA groups=2).
Each core computes a partial (T, D) output (its 2 heads' slice of the
o-projection contraction); the host sums the 8 partials (TP unshard).

v2 layout strategy per core (all matmul operands bf16, PSUM fp32):
  - x_all^T [D, S] fed bf16; ONE big DMA per 512-token chunk
    ([128, 16 d-tiles x 512] staging tile) instead of 16 small ones.
  - fused QKV projection: one weight tile [wk|wv|wq0|wq1] per d-tile;
    noise-token chunks project q in the same matmul group as k/v
    (512-wide moving operand), so x is loaded exactly once.
  - RMSNorm stats batched per chunk (one Sqrt, one reciprocal for all
    tiles); RoPE in bf16 via on-device sin/cos tables (Cody-Waite range
    reduction + ACT Sin), tables built once for all 48 token tiles.
  - attention in [s, t] orientation: scores^T = kT.T @ qT; exp on ACT
    (scale=1/sqrt(H) folded) straight to bf16; no max subtraction
    (|score| <= sqrt(H)*1.1^2 ~ 13.7, exp safe in fp32/bf16); row-sums
    via ones-matmul accumulated in PSUM; A@V accumulates over s-tiles in
    PSUM with V in natural [s, h] bf16 layout.
  - softmax division deferred past the o-projection (denominator is
    constant along the contraction): per-(head,t-chunk) row-sums are
    copied to SBUF, gathered into per-partition columns by one tiny
    SBUF->SBUF DMA each, reciprocal'd once, applied as per-partition
    scalars on the o-projection output.
"""

import sys

for _p in ("/opt/trn_rl_repo", "/root/.axon_site/_ro/trn_rl_repo"):
    if _p not in sys.path:
        sys.path.append(_p)

import math
import numpy as np
import ml_dtypes

import concourse.bass as bass
import concourse.tile as tile
from concourse import bacc
from concourse import mybir
from concourse.bass_utils import run_bass_kernel_spmd
from concourse.masks import make_identity

D = 2048
N_HEADS = 16
K_HEADS = 8
H = 128
T_NOISE = 2048
T_CTX = 4096
S_ALL = T_CTX + T_NOISE          # 6144
EPS = 1e-6
ROPE_THETA = 1e6
N_CORES = 8
HEADS_PER_CORE = N_HEADS // N_CORES   # 2

P = 128                       # partition dim
S_TILES = S_ALL // P          # 48
T_TILES = T_NOISE // P        # 16
NOISE_TILE0 = T_CTX // P      # 32  (noise tokens are s-tiles 32..47)
D_TILES = D // P              # 16
FREE = 512                    # moving free-dim chunk
T_CHUNKS = T_NOISE // FREE    # 4
S_CHUNKS = S_ALL // FREE      # 12
D_CHUNKS = D // FREE          # 4
NOISE_CHUNK0 = T_CTX // FREE  # 8   (chunks 8..11 are noise tokens)

F32 = mybir.dt.float32
BF16 = mybir.dt.bfloat16
MM_DT = BF16                  # dtype for all matmul operands

TWO_PI = 2.0 * math.pi
INV_SQRT_H = 1.0 / math.sqrt(H)
MULT = mybir.AluOpType.mult
ADD = mybir.AluOpType.add

_CACHE = {}


def _build_program(reps=1):
    """Build the single-core SPMD bass program. Returns (nc, out_name)."""
    nc = bacc.Bacc("TRN2", target_bir_lowering=False, debug=False,
                   num_devices=N_CORES)

    xT = nc.dram_tensor("xT", [D, S_ALL], MM_DT, kind="ExternalInput").ap()
    wqkv = nc.dram_tensor("wqkv", [D, 4 * H], MM_DT, kind="ExternalInput").ap()
    wo = nc.dram_tensor("wo", [HEADS_PER_CORE, H, D], MM_DT,
                        kind="ExternalInput").ap()
    posr = nc.dram_tensor("posr", [S_TILES, P, 1], F32,
                          kind="ExternalInput").ap()
    invfb = nc.dram_tensor("invfb", [P, H // 2], F32,
                           kind="ExternalInput").ap()
    qscaleb = nc.dram_tensor("qscaleb", [P, H], F32,
                             kind="ExternalInput").ap()
    kscaleb = nc.dram_tensor("kscaleb", [P, H], F32,
                             kind="ExternalInput").ap()
    onesb = nc.dram_tensor("onesb", [P, 1], MM_DT, kind="ExternalInput").ap()
    out = nc.dram_tensor("out", [T_NOISE, D], F32, kind="ExternalOutput").ap()

    with tile.TileContext(nc) as tc:
        for rep in range(reps):
            _emit(nc, tc, xT, wqkv, wo, posr, invfb, qscaleb, kscaleb,
                  onesb, out, pfx=f"r{rep}_")
    nc.compile()
    return nc, "out"


def _emit(nc, tc, xT, wqkv, wo, posr, invfb, qscaleb, kscaleb, onesb, out,
          pfx=""):
    import contextlib
    ctx = contextlib.ExitStack()
    half = H // 2
    with ctx:
        const = ctx.enter_context(tc.tile_pool(name=pfx + "const", bufs=1))
        persist = ctx.enter_context(tc.tile_pool(name=pfx + "persist", bufs=1))

        # ---- constants ----
        ident = const.tile([P, P], MM_DT, tag="ident")
        make_identity(nc, ident[:])
        ones = const.tile([P, 1], MM_DT, tag="ones")
        nc.sync.dma_start(ones[:], onesb[:])
        invf_sb = const.tile([P, half], F32, tag="invf")
        nc.sync.dma_start(invf_sb[:], invfb[:])
        qsc_sb = const.tile([P, H], F32, tag="qsc")
        nc.sync.dma_start(qsc_sb[:], qscaleb[:])
        ksc_sb = const.tile([P, H], F32, tag="ksc")
        nc.sync.dma_start(ksc_sb[:], kscaleb[:])
        eps_col = const.tile([P, 1], F32, tag="eps")
        nc.vector.memset(eps_col[:], EPS)
        pos_sb = const.tile([P, S_TILES], F32, tag="pos")
        nc.sync.dma_start(pos_sb[:], posr.rearrange("t p o -> p (t o)"))

        # weights: one DMA for the fused [wk|wv|wq0|wq1] tile, two for wo
        wqkv_sb = const.tile([P, D_TILES * 4 * H], MM_DT, tag="wqkv")
        nc.sync.dma_start(wqkv_sb[:],
                          wqkv.rearrange("(d p) c -> p d c", p=P))
        wo_sb = [const.tile([P, D], MM_DT, tag=f"wo{h}", name=f"wos{h}")
                 for h in range(HEADS_PER_CORE)]
        for h in range(HEADS_PER_CORE):
            nc.sync.dma_start(wo_sb[h][:], wo[h])

        # ---- persistent activations ----
        sin_all = persist.tile([P, S_TILES * half], MM_DT, tag="sin")
        cos_all = persist.tile([P, S_TILES * half], MM_DT, tag="cos")
        kT_sb = persist.tile([P, S_ALL], MM_DT, tag="kT")
        v_sb = persist.tile([P, S_ALL], MM_DT, tag="v")     # [s-tile, h] blocks
        qT_sb = persist.tile([P, HEADS_PER_CORE * T_NOISE], MM_DT, tag="qT")
        oT_sb = persist.tile([P, HEADS_PER_CORE * T_NOISE], MM_DT, tag="oT")
        rsum = persist.tile([1, HEADS_PER_CORE * T_NOISE], F32, tag="rsum")
        rcol = persist.tile([P, HEADS_PER_CORE * T_TILES], F32, tag="rcol")

        # ---- RoPE sin/cos tables for all 48 token tiles (bf16 out) ----
        CW1, CW2, CW3 = 6.28125, 0.0019353071693331003, 1.0253131677018246e-11
        HGRP = S_TILES // 2
        HW_ = HGRP * half
        with tc.tile_pool(name=pfx + "ropebuild", bufs=1) as rp:
            for g in range(2):
                ang = rp.tile([P, HW_], F32, tag="ang", name="ang")
                kq = rp.tile([P, HW_], F32, tag="kq", name="kq")
                ki = rp.tile([P, HW_], mybir.dt.int32, tag="ki", name="ki")
                wrap = rp.tile([P, HW_], F32, tag="wrap", name="wrap")
                for j in range(HGRP):
                    si = g * HGRP + j
                    nc.vector.tensor_scalar_mul(
                        ang[:, j * half:(j + 1) * half], invf_sb[:, :],
                        pos_sb[:, si:si + 1])
                nc.vector.tensor_scalar_mul(kq[:], ang[:], 1.0 / TWO_PI)
                nc.vector.tensor_copy(ki[:], kq[:])
                nc.vector.tensor_copy(kq[:], ki[:])
                nc.vector.cody_waite_cascade(ang[:], ang[:], kq[:],
                                             CW1, CW2, CW3)
                dst = slice(g * HW_, (g + 1) * HW_)
                nc.vector.add_range_wrap(wrap[:], ang[:], 0.0, math.pi, TWO_PI)
                nc.scalar.activation(sin_all[:, dst], wrap[:],
                                     mybir.ActivationFunctionType.Sin)
                nc.vector.add_range_wrap(wrap[:], ang[:], math.pi / 2, math.pi,
                                         TWO_PI)
                nc.scalar.activation(cos_all[:, dst], wrap[:],
                                     mybir.ActivationFunctionType.Sin)

        def norm_rope_transpose(src_psum, scale_sb, si, dst_sb, rinv_col,
                                work, psum_t):
            """src_psum [P(tok),H] fp32 -> rms-norm*scale -> rope (bf16)
            -> transpose -> dst_sb [P(h), 128 tok] bf16."""
            xn = work.tile([P, H], MM_DT, tag="xn")
            nc.vector.scalar_tensor_tensor(
                xn[:], src_psum, rinv_col, scale_sb[:], MULT, MULT)
            co = cos_all[:, si * half:(si + 1) * half]
            sn = sin_all[:, si * half:(si + 1) * half]
            x1 = xn[:, 0:half]
            x2 = xn[:, half:H]
            t1 = work.tile([P, half], MM_DT, tag="t1")
            t2 = work.tile([P, half], MM_DT, tag="t2")
            xr = work.tile([P, H], MM_DT, tag="xr")
            nc.vector.tensor_mul(t1[:], x1, co)
            nc.vector.tensor_mul(t2[:], x2, sn)
            nc.vector.tensor_sub(xr[:, 0:half], t1[:], t2[:])
            nc.vector.tensor_mul(t1[:], x2, co)
            nc.vector.tensor_mul(t2[:], x1, sn)
            nc.vector.tensor_add(xr[:, half:H], t1[:], t2[:])
            pt = psum_t.tile([P, P], MM_DT, tag="pt")
            nc.tensor.transpose(pt[:], xr[:], ident[:])
            nc.vector.tensor_copy(dst_sb, pt[:])

        # ---- Phase A: fused QKV projection + norm/rope for all chunks ----
        with tc.tile_pool(name=pfx + "pa_x", bufs=3) as xp, \
             tc.tile_pool(name=pfx + "pa_ps", bufs=6, space="PSUM") as pa, \
             tc.tile_pool(name=pfx + "pa_pt", bufs=2, space="PSUM") as pst, \
             tc.tile_pool(name=pfx + "pa_w", bufs=3) as work:
            for sc in range(S_CHUNKS):
                noise = sc >= NOISE_CHUNK0
                w_w = 4 * H if noise else 2 * H
                xt = xp.tile([P, D_TILES * FREE], MM_DT, tag="xc",
                             name="xchunk")
                nc.sync.dma_start(
                    xt[:], xT.rearrange("(d p) s -> p d s", p=P)[
                        :, :, sc * FREE:(sc + 1) * FREE])
                slots = [pa.tile([P, 4 * H], F32, tag="pj", name=f"pj{j}")
                         for j in range(4)]
                for d in range(D_TILES):
                    for j in range(4):
                        nc.tensor.matmul(
                            slots[j][:, 0:w_w],
                            xt[:, d * FREE + j * P:d * FREE + (j + 1) * P],
                            wqkv_sb[:, d * 4 * H:d * 4 * H + w_w],
                            start=(d == 0), stop=(d == D_TILES - 1))
                # batched RMS stats: k per j (cols 0..3), q0 (4..7), q1 (8..11)
                ncols = 12 if noise else 4
                ssq = work.tile([P, 12], F32, tag="ssq")
                rms = work.tile([P, 12], F32, tag="rms")
                rinv = work.tile([P, 12], F32, tag="rinv")
                sq = [work.tile([P, H], F32, tag="sq", name=f"sq{j}")
                      for j in range(4)]
                for j in range(4):
                    nc.scalar.activation(
                        sq[j][:], slots[j][:, 0:H],
                        mybir.ActivationFunctionType.Square,
                        accum_out=ssq[:, j:j + 1])
                if noise:
                    for j in range(4):
                        nc.scalar.activation(
                            sq[j][:], slots[j][:, 2 * H:3 * H],
                            mybir.ActivationFunctionType.Square,
                            accum_out=ssq[:, 4 + j:5 + j])
                    for j in range(4):
                        nc.scalar.activation(
                            sq[j][:], slots[j][:, 3 * H:4 * H],
                            mybir.ActivationFunctionType.Square,
                            accum_out=ssq[:, 8 + j:9 + j])
                nc.scalar.activation(rms[:, 0:ncols], ssq[:, 0:ncols],
                                     mybir.ActivationFunctionType.Sqrt,
                                     bias=eps_col[:], scale=1.0 / H)
                nc.vector.reciprocal(rinv[:, 0:ncols], rms[:, 0:ncols])
                for j in range(4):
                    si = sc * 4 + j
                    nc.vector.tensor_copy(v_sb[:, si * P:(si + 1) * P],
                                          slots[j][:, H:2 * H])
                    norm_rope_transpose(slots[j][:, 0:H], ksc_sb, si,
                                        kT_sb[:, si * P:(si + 1) * P],
                                        rinv[:, j:j + 1], work, pst)
                    if noise:
                        ti = (sc - NOISE_CHUNK0) * 4 + j
                        for hh in range(HEADS_PER_CORE):
                            norm_rope_transpose(
                                slots[j][:, (2 + hh) * H:(3 + hh) * H],
                                qsc_sb, NOISE_TILE0 + ti,
                                qT_sb[:, hh * T_NOISE + ti * P:
                                      hh * T_NOISE + (ti + 1) * P],
                                rinv[:, 4 + 4 * hh + j:5 + 4 * hh + j],
                                work, pst)

        # ---- Phase C: attention ----
        PAIR = 2 * FREE   # exp processes two score banks at once
        with tc.tile_pool(name=pfx + "pc_sc", bufs=2, space="PSUM") as psc, \
             tc.tile_pool(name=pfx + "pc_av", bufs=2, space="PSUM") as pav, \
             tc.tile_pool(name=pfx + "pc_r", bufs=2, space="PSUM") as pr, \
             tc.tile_pool(name=pfx + "pc_exp", bufs=3) as pexp:
            for hh in range(HEADS_PER_CORE):
                for tch in range(T_CHUNKS):
                    qslice = qT_sb[:, hh * T_NOISE + tch * FREE:
                                   hh * T_NOISE + (tch + 1) * FREE]
                    av = pav.tile([P, FREE], F32, tag="av")
                    rr = pr.tile([1, FREE], F32, tag="rr")
                    for sp in range(S_TILES // 2):
                        sc_ps = psc.tile([P, PAIR], F32, tag="sc")
                        ex = pexp.tile([P, PAIR], MM_DT, tag="ex")
                        for u in range(2):
                            si = sp * 2 + u
                            nc.tensor.matmul(
                                sc_ps[:, u * FREE:(u + 1) * FREE],
                                kT_sb[:, si * P:(si + 1) * P], qslice,
                                start=True, stop=True)
                        nc.scalar.activation(ex[:], sc_ps[:],
                                             mybir.ActivationFunctionType.Exp,
                                             scale=INV_SQRT_H)
                        for u in range(2):
                            si = sp * 2 + u
                            first = si == 0
                            last = si == S_TILES - 1
                            nc.tensor.matmul(
                                av[:], v_sb[:, si * P:(si + 1) * P],
                                ex[:, u * FREE:(u + 1) * FREE],
                                start=first, stop=last)
                            nc.tensor.matmul(
                                rr[:], ones[:],
                                ex[:, u * FREE:(u + 1) * FREE],
                                start=first, stop=last)
                    cbase = hh * T_NOISE + tch * FREE
                    nc.vector.tensor_copy(
                        oT_sb[:, cbase:cbase + FREE], av[:])
                    nc.vector.tensor_copy(
                        rsum[0:1, cbase:cbase + FREE], rr[:])
                    # gather this chunk's row-sums into per-partition columns
                    for t4 in range(FREE // P):
                        col = hh * T_TILES + tch * (FREE // P) + t4
                        nc.sync.dma_start(
                            rcol[:, col:col + 1],
                            rsum[0:1, cbase + t4 * P:cbase + (t4 + 1) * P])
            nc.vector.reciprocal(rcol[:], rcol[:])

        # ---- Phase D: o-projection + deferred softmax normalization ----
        with tc.tile_pool(name=pfx + "pd_ps", bufs=2, space="PSUM") as pso, \
             tc.tile_pool(name=pfx + "pd_w", bufs=2) as dwork, \
             tc.tile_pool(name=pfx + "pd_o", bufs=2) as ostage:
            for ti in range(T_TILES):
                ot = ostage.tile([P, D], F32, tag="ot", name="otile")
                for dc in range(D_CHUNKS):
                    po = [pso.tile([P, FREE], F32, tag=f"po{h}", name=f"po{h}")
                          for h in range(HEADS_PER_CORE)]
                    for h in range(HEADS_PER_CORE):
                        nc.tensor.matmul(
                            po[h][:],
                            oT_sb[:, h * T_NOISE + ti * P:
                                  h * T_NOISE + (ti + 1) * P],
                            wo_sb[h][:, dc * FREE:(dc + 1) * FREE],
                            start=True, stop=True)
                    tmp = dwork.tile([P, FREE], F32, tag="tmp")
                    nc.scalar.activation(
                        tmp[:], po[1][:], mybir.ActivationFunctionType.Copy,
                        scale=rcol[:, T_TILES + ti:T_TILES + ti + 1])
                    nc.vector.scalar_tensor_tensor(
                        ot[:, dc * FREE:(dc + 1) * FREE], po[0][:],
                        rcol[:, ti:ti + 1], tmp[:], MULT, ADD)
                nc.sync.dma_start(out[ti * P:(ti + 1) * P, :], ot[:])


def _get_program(reps=1):
    key = f"prog{reps}"
    if key not in _CACHE:
        _CACHE[key] = _build_program(reps)
    return _CACHE[key]


def prepare_in_maps(x_noise, target_hidden, Wq, Wk, Wv, Wo, q_scale, k_scale,
                    noise_positions, ctx_positions):
    x_noise = np.asarray(x_noise, dtype=np.float32)
    target_hidden = np.asarray(target_hidden, dtype=np.float32)
    Wq = np.asarray(Wq, dtype=np.float32)
    Wk = np.asarray(Wk, dtype=np.float32)
    Wv = np.asarray(Wv, dtype=np.float32)
    Wo = np.asarray(Wo, dtype=np.float32)
    q_scale = np.asarray(q_scale, dtype=np.float32)
    k_scale = np.asarray(k_scale, dtype=np.float32)

    x_all = np.concatenate([target_hidden, x_noise], axis=0)       # (S, D)
    xT = np.ascontiguousarray(x_all.T.astype(ml_dtypes.bfloat16))  # (D, S)
    pos_all = np.concatenate(
        [np.asarray(ctx_positions), np.asarray(noise_positions)]
    ).astype(np.float32)
    posr = np.ascontiguousarray(pos_all.reshape(S_TILES, P, 1))
    half = H // 2
    inv_freq = (ROPE_THETA ** (-np.arange(half, dtype=np.float32) * 2.0 / H)
                ).astype(np.float32)
    invfb = np.ascontiguousarray(np.broadcast_to(inv_freq, (P, half)))
    qscaleb = np.ascontiguousarray(np.broadcast_to(q_scale, (P, H)))
    kscaleb = np.ascontiguousarray(np.broadcast_to(k_scale, (P, H)))

    in_maps = []
    for c in range(N_CORES):
        wqkv = np.ascontiguousarray(np.concatenate(
            [Wk[:, c, :], Wv[:, c, :],
             Wq[:, 2 * c, :], Wq[:, 2 * c + 1, :]],
            axis=1).astype(ml_dtypes.bfloat16))                     # (D, 512)
        wo_c = np.ascontiguousarray(
            Wo[c * HEADS_PER_CORE:(c + 1) * HEADS_PER_CORE]
            .astype(ml_dtypes.bfloat16))                            # (2,128,D)
        in_maps.append({
            "xT": xT, "wqkv": wqkv, "wo": wo_c,
            "posr": posr, "invfb": invfb,
            "qscaleb": qscaleb, "kscaleb": kscaleb,
            "onesb": np.ones((P, 1), dtype=ml_dtypes.bfloat16),
        })
    return in_maps


def kernel(**inputs):
    in_maps = prepare_in_maps(**inputs)
    nc, out_name = _get_program()
    res = run_bass_kernel_spmd(nc, in_maps, core_ids=list(range(N_CORES)))
    acc = np.zeros((T_NOISE, D), dtype=np.float32)
    for r in res.results:
        acc += r[out_name]
    return acc


def run_traced(inputs, **kw):
    """Run once with NTFF tracing; returns BassKernelResults (exec_time_ns)."""
    in_maps = prepare_in_maps(**inputs)
    nc, out_name = _get_program()
    return run_bass_kernel_spmd(nc, in_maps, core_ids=list(range(N_CORES)),
                                trace=True, **kw)


# revision 9
# speedup vs baseline: 2.0730x; 1.4383x over previous
"""DFlashAttention Trainium2 kernel (8-core tensor-parallel over attention heads).

Shapes (hardcoded): D=2048, N=16 q-heads, K=8 kv-heads, H=128,
T_NOISE=2048 (query tokens), T_CTX=4096, S=6144 (kv tokens).

Sharding: core c owns q-heads {2c, 2c+1} and kv-head c (GQA groups=2).
Each core computes a partial (T, D) output (its 2 heads' slice of the
o-projection contraction); the host sums the 8 partials (TP unshard).

v3 layout strategy per core (all matmul operands bf16, PSUM fp32):
  - x_all^T [D, S] fed bf16; ONE big DMA per 512-token chunk
    ([128, 16 d-tiles x 512] staging tile).
  - fused QKV projection: one weight tile [wk|wv|wq0|wq1] per d-tile;
    noise-token chunks project q in the same matmul group as k/v
    (512-wide moving operand), so x is loaded exactly once. Chunks are
    emitted noise/ctx interleaved and each chunk's [token,H]->[H,token]
    transposes are deferred behind the next chunk's matmuls so the PE
    never waits on the vector engine (in-order queue hazard).
  - RMSNorm stats batched per chunk (one Sqrt, one reciprocal for all
    tiles); RoPE in bf16 with HOST-precomputed sin/cos tables (two DMA
    loads, no on-device table build).
  - attention in [s, t] orientation, software-pipelined: scores^T for
    pair i+1 are issued before A@V/row-sum of pair i so the PE streams
    through exp latency. exp on ACT (scale=1/sqrt(H) folded) straight
    to bf16; no max subtraction (|score| <= 13.7, exp safe in fp32).
  - softmax normalization applied at the oT copy: row-sums (ones-matmul
    in PSUM) -> SBUF -> GpSimd partition_broadcast -> reciprocal ->
    one tensor_mul. Phase D is then a pure 2-head accumulating
    o-projection plus a PSUM->SBUF copy and one 1 MB store per t-tile.
"""

import sys

for _p in ("/opt/trn_rl_repo", "/root/.axon_site/_ro/trn_rl_repo"):
    if _p not in sys.path:
        sys.path.append(_p)

import math
import numpy as np
import ml_dtypes

import concourse.bass as bass
import concourse.tile as tile
from concourse import bacc
from concourse import mybir
from concourse.bass_utils import run_bass_kernel_spmd
from concourse.masks import make_identity

D = 2048
N_HEADS = 16
K_HEADS = 8
H = 128
T_NOISE = 2048
T_CTX = 4096
S_ALL = T_CTX + T_NOISE          # 6144
EPS = 1e-6
ROPE_THETA = 1e6
N_CORES = 8
HEADS_PER_CORE = N_HEADS // N_CORES   # 2

P = 128                       # partition dim
S_TILES = S_ALL // P          # 48
T_TILES = T_NOISE // P        # 16
NOISE_TILE0 = T_CTX // P      # 32  (noise tokens are s-tiles 32..47)
D_TILES = D // P              # 16
FREE = 512                    # moving free-dim chunk
T_CHUNKS = T_NOISE // FREE    # 4
S_CHUNKS = S_ALL // FREE      # 12
D_CHUNKS = D // FREE          # 4
NOISE_CHUNK0 = T_CTX // FREE  # 8   (chunks 8..11 are noise tokens)

F32 = mybir.dt.float32
BF16 = mybir.dt.bfloat16
MM_DT = BF16                  # dtype for all matmul operands

INV_SQRT_H = 1.0 / math.sqrt(H)
MULT = mybir.AluOpType.mult

_CACHE = {}

# phase-A chunk emission order: noise/ctx interleaved so adjacent chunks
# never need two full noise PSUM slot sets at once, and the PE always has
# a fresh chunk's matmuls to run while the previous chunk's norm drains.
CHUNK_ORDER = [8, 0, 9, 1, 10, 2, 11, 3, 4, 5, 6, 7]


def _build_program(reps=1):
    """Build the single-core SPMD bass program. Returns (nc, out_name)."""
    nc = bacc.Bacc("TRN2", target_bir_lowering=False, debug=False,
                   num_devices=N_CORES)

    xT = nc.dram_tensor("xT", [D, S_ALL], MM_DT, kind="ExternalInput").ap()
    wqkv = nc.dram_tensor("wqkv", [D, 4 * H], MM_DT, kind="ExternalInput").ap()
    wo = nc.dram_tensor("wo", [HEADS_PER_CORE, H, D], MM_DT,
                        kind="ExternalInput").ap()
    sinb = nc.dram_tensor("sinb", [S_ALL, H // 2], MM_DT,
                          kind="ExternalInput").ap()
    cosb = nc.dram_tensor("cosb", [S_ALL, H // 2], MM_DT,
                          kind="ExternalInput").ap()
    qscaleb = nc.dram_tensor("qscaleb", [P, H], F32,
                             kind="ExternalInput").ap()
    kscaleb = nc.dram_tensor("kscaleb", [P, H], F32,
                             kind="ExternalInput").ap()
    out = nc.dram_tensor("out", [T_NOISE, D], F32, kind="ExternalOutput").ap()

    with tile.TileContext(nc) as tc:
        for rep in range(reps):
            _emit(nc, tc, xT, wqkv, wo, sinb, cosb, qscaleb, kscaleb,
                  out, pfx=f"r{rep}_")
    nc.compile()
    return nc, "out"


def _emit(nc, tc, xT, wqkv, wo, sinb, cosb, qscaleb, kscaleb, out, pfx=""):
    import contextlib
    ctx = contextlib.ExitStack()
    half = H // 2
    xTr = xT.rearrange("(d p) s -> p d s", p=P)
    with ctx:
        const = ctx.enter_context(tc.tile_pool(name=pfx + "const", bufs=1))
        persist = ctx.enter_context(tc.tile_pool(name=pfx + "persist", bufs=1))

        # ---- weights first (phase A needs them immediately) ----
        wqkv_sb = const.tile([P, D_TILES * 4 * H], MM_DT, tag="wqkv")
        nc.sync.dma_start(wqkv_sb[:],
                          wqkv.rearrange("(d p) c -> p d c", p=P))

        # ---- persistent activations ----
        sin_all = persist.tile([P, S_TILES * half], MM_DT, tag="sin")
        cos_all = persist.tile([P, S_TILES * half], MM_DT, tag="cos")
        kT_sb = persist.tile([P, S_ALL], MM_DT, tag="kT")
        v_sb = persist.tile([P, S_ALL], MM_DT, tag="v")     # [s-tile, h] blocks
        qT_sb = persist.tile([P, HEADS_PER_CORE * T_NOISE], MM_DT, tag="qT")
        oT_sb = persist.tile([P, HEADS_PER_CORE * T_NOISE], MM_DT, tag="oT")

        # ---- other constants ----
        ident = const.tile([P, P], MM_DT, tag="ident")
        make_identity(nc, ident[:])
        ones = const.tile([P, 1], MM_DT, tag="ones")
        nc.vector.memset(ones[:], 1.0)
        eps_col = const.tile([P, 1], F32, tag="eps")
        nc.vector.memset(eps_col[:], EPS)
        nc.sync.dma_start(sin_all[:],
                          sinb.rearrange("(t p) f -> p t f", p=P))
        nc.sync.dma_start(cos_all[:],
                          cosb.rearrange("(t p) f -> p t f", p=P))
        qsc_sb = const.tile([P, H], F32, tag="qsc")
        nc.sync.dma_start(qsc_sb[:], qscaleb[:])
        ksc_sb = const.tile([P, H], F32, tag="ksc")
        nc.sync.dma_start(ksc_sb[:], kscaleb[:])
        wo_sb = [const.tile([P, D], MM_DT, tag=f"wo{h}", name=f"wos{h}")
                 for h in range(HEADS_PER_CORE)]
        for h in range(HEADS_PER_CORE):
            nc.sync.dma_start(wo_sb[h][:], wo[h])

        def norm_rope(src_psum, scale_sb, si, rinv_col, work):
            """src_psum [P(tok),H] fp32 -> rms-norm*scale -> rope -> bf16
            [P(tok), H] tile (returned; transpose deferred)."""
            xn = work.tile([P, H], MM_DT, tag="xn")
            nc.vector.scalar_tensor_tensor(
                xn[:], src_psum, rinv_col, scale_sb[:], MULT, MULT)
            co = cos_all[:, si * half:(si + 1) * half]
            sn = sin_all[:, si * half:(si + 1) * half]
            x1 = xn[:, 0:half]
            x2 = xn[:, half:H]
            t1 = work.tile([P, half], MM_DT, tag="t1")
            t2 = work.tile([P, half], MM_DT, tag="t2")
            xr = work.tile([P, H], MM_DT, tag="xr")
            nc.vector.tensor_mul(t1[:], x1, co)
            nc.vector.tensor_mul(t2[:], x2, sn)
            nc.vector.tensor_sub(xr[:, 0:half], t1[:], t2[:])
            nc.vector.tensor_mul(t1[:], x2, co)
            nc.vector.tensor_mul(t2[:], x1, sn)
            nc.vector.tensor_add(xr[:, half:H], t1[:], t2[:])
            return xr

        # ---- Phase A: fused QKV projection + norm/rope for all chunks ----
        with tc.tile_pool(name=pfx + "pa_x", bufs=3) as xp, \
             tc.tile_pool(name=pfx + "pa_ps", bufs=4, space="PSUM") as pa, \
             tc.tile_pool(name=pfx + "pa_ps2", bufs=2, space="PSUM") as pa2, \
             tc.tile_pool(name=pfx + "pa_pt", bufs=2, space="PSUM") as pst, \
             tc.tile_pool(name=pfx + "pa_w", bufs=4) as work, \
             tc.tile_pool(name=pfx + "pa_xr", bufs=16) as xrp:
            pending = []   # deferred transposes: (xr_tile, dst_sb_slice)

            def flush_pending():
                for xr, dst in pending:
                    pt = pst.tile([P, P], MM_DT, tag="pt")
                    nc.tensor.transpose(pt[:], xr[:], ident[:])
                    nc.vector.tensor_copy(dst, pt[:])
                pending.clear()

            for sc in CHUNK_ORDER:
                noise = sc >= NOISE_CHUNK0
                w_w = 4 * H if noise else 2 * H
                xt = xp.tile([P, D_TILES * FREE], MM_DT, tag="xc",
                             name="xchunk")
                nc.sync.dma_start(
                    xt[:], xTr[:, :, sc * FREE:(sc + 1) * FREE])
                if noise:
                    slots = [pa.tile([P, w_w], F32, tag="pjn", name=f"pjn{j}")
                             for j in range(4)]
                else:
                    # pack two 256-wide ctx outputs per PSUM bank
                    banks = [pa2.tile([P, 2 * w_w], F32, tag="pjc",
                                      name=f"pjc{b}") for b in range(2)]
                    slots = [banks[j // 2][:, (j % 2) * w_w:(j % 2 + 1) * w_w]
                             for j in range(4)]
                # j outer, d inner: each j's accumulation group runs start..
                # stop without another group's start= clearing its bank's
                # has_written bits (two ctx groups share one PSUM bank).
                for j in range(4):
                    for d in range(D_TILES):
                        nc.tensor.matmul(
                            slots[j][:],
                            xt[:, d * FREE + j * P:d * FREE + (j + 1) * P],
                            wqkv_sb[:, d * 4 * H:d * 4 * H + w_w],
                            start=(d == 0), stop=(d == D_TILES - 1))
                flush_pending()   # prev chunk's transposes, inputs now ready
                # batched RMS stats: k per j (cols 0..3), q0 (4..7), q1 (8..11)
                ncols = 12 if noise else 4
                ssq = work.tile([P, 12], F32, tag="ssq")
                rms = work.tile([P, 12], F32, tag="rms")
                rinv = work.tile([P, 12], F32, tag="rinv")
                sq = [work.tile([P, H], F32, tag="sq", name=f"sq{j}")
                      for j in range(4)]
                for j in range(4):
                    nc.scalar.activation(
                        sq[j][:], slots[j][:, 0:H],
                        mybir.ActivationFunctionType.Square,
                        accum_out=ssq[:, j:j + 1])
                if noise:
                    for hh in range(HEADS_PER_CORE):
                        for j in range(4):
                            nc.scalar.activation(
                                sq[j][:], slots[j][:, (2 + hh) * H:(3 + hh) * H],
                                mybir.ActivationFunctionType.Square,
                                accum_out=ssq[:, 4 + 4 * hh + j:5 + 4 * hh + j])
                nc.scalar.activation(rms[:, 0:ncols], ssq[:, 0:ncols],
                                     mybir.ActivationFunctionType.Sqrt,
                                     bias=eps_col[:], scale=1.0 / H)
                nc.vector.reciprocal(rinv[:, 0:ncols], rms[:, 0:ncols])
                for j in range(4):
                    si = sc * 4 + j
                    nc.vector.tensor_copy(v_sb[:, si * P:(si + 1) * P],
                                          slots[j][:, H:2 * H])
                    xr = norm_rope(slots[j][:, 0:H], ksc_sb, si,
                                   rinv[:, j:j + 1], xrp)
                    pending.append((xr, kT_sb[:, si * P:(si + 1) * P]))
                    if noise:
                        ti = (sc - NOISE_CHUNK0) * 4 + j
                        for hh in range(HEADS_PER_CORE):
                            xr = norm_rope(
                                slots[j][:, (2 + hh) * H:(3 + hh) * H],
                                qsc_sb, NOISE_TILE0 + ti,
                                rinv[:, 4 + 4 * hh + j:5 + 4 * hh + j], xrp)
                            pending.append(
                                (xr, qT_sb[:, hh * T_NOISE + ti * P:
                                           hh * T_NOISE + (ti + 1) * P]))
            flush_pending()

        # ---- Phase C: attention (software-pipelined) ----
        PAIR = 2 * FREE   # exp processes two score banks at once
        NP_ = S_TILES // 2
        with tc.tile_pool(name=pfx + "pc_sc", bufs=2, space="PSUM") as psc, \
             tc.tile_pool(name=pfx + "pc_av", bufs=2, space="PSUM") as pav, \
             tc.tile_pool(name=pfx + "pc_r", bufs=2, space="PSUM") as pr, \
             tc.tile_pool(name=pfx + "pc_exp", bufs=3) as pexp, \
             tc.tile_pool(name=pfx + "pc_w", bufs=2) as cwork:
            for hh in range(HEADS_PER_CORE):
                for tch in range(T_CHUNKS):
                    qslice = qT_sb[:, hh * T_NOISE + tch * FREE:
                                   hh * T_NOISE + (tch + 1) * FREE]
                    av = pav.tile([P, FREE], F32, tag="av")
                    rr = pr.tile([1, FREE], F32, tag="rr")

                    def scores_exp(sp):
                        sc_ps = psc.tile([P, PAIR], F32, tag="sc")
                        ex = pexp.tile([P, PAIR], MM_DT, tag="ex")
                        for u in range(2):
                            si = sp * 2 + u
                            nc.tensor.matmul(
                                sc_ps[:, u * FREE:(u + 1) * FREE],
                                kT_sb[:, si * P:(si + 1) * P], qslice,
                                start=True, stop=True)
                        nc.scalar.activation(ex[:], sc_ps[:],
                                             mybir.ActivationFunctionType.Exp,
                                             scale=INV_SQRT_H)
                        return ex

                    def av_rsum(sp, ex):
                        for u in range(2):
                            si = sp * 2 + u
                            first = si == 0
                            last = si == S_TILES - 1
                            nc.tensor.matmul(
                                av[:], v_sb[:, si * P:(si + 1) * P],
                                ex[:, u * FREE:(u + 1) * FREE],
                                start=first, stop=last)
                            nc.tensor.matmul(
                                rr[:], ones[:],
                                ex[:, u * FREE:(u + 1) * FREE],
                                start=first, stop=last)

                    ex_prev = scores_exp(0)
                    for sp in range(1, NP_):
                        ex_cur = scores_exp(sp)
                        av_rsum(sp - 1, ex_prev)
                        ex_prev = ex_cur
                    av_rsum(NP_ - 1, ex_prev)

                    # normalize: oT = av / rowsum (broadcast recip over h)
                    cbase = hh * T_NOISE + tch * FREE
                    rrow = cwork.tile([1, FREE], F32, tag="rrow")
                    nc.vector.tensor_copy(rrow[:], rr[:])
                    rb = cwork.tile([P, FREE], F32, tag="rb")
                    nc.gpsimd.partition_broadcast(rb[:], rrow[:])
                    nc.vector.reciprocal(rb[:], rb[:])
                    nc.vector.tensor_mul(oT_sb[:, cbase:cbase + FREE],
                                         av[:], rb[:])

        # ---- Phase D: o-projection (2-head accumulate) + store ----
        with tc.tile_pool(name=pfx + "pd_ps", bufs=3, space="PSUM") as pso, \
             tc.tile_pool(name=pfx + "pd_o", bufs=3) as ostage:
            for ti in range(T_TILES):
                ot = ostage.tile([P, D], F32, tag="ot", name="otile")
                for dc in range(D_CHUNKS):
                    po = pso.tile([P, FREE], F32, tag="po")
                    for h in range(HEADS_PER_CORE):
                        nc.tensor.matmul(
                            po[:],
                            oT_sb[:, h * T_NOISE + ti * P:
                                  h * T_NOISE + (ti + 1) * P],
                            wo_sb[h][:, dc * FREE:(dc + 1) * FREE],
                            start=(h == 0), stop=(h == HEADS_PER_CORE - 1))
                    nc.vector.tensor_copy(ot[:, dc * FREE:(dc + 1) * FREE],
                                          po[:])
                nc.sync.dma_start(out[ti * P:(ti + 1) * P, :], ot[:])


def _get_program(reps=1):
    key = f"prog{reps}"
    if key not in _CACHE:
        _CACHE[key] = _build_program(reps)
    return _CACHE[key]


def prepare_in_maps(x_noise, target_hidden, Wq, Wk, Wv, Wo, q_scale, k_scale,
                    noise_positions, ctx_positions):
    x_noise = np.asarray(x_noise, dtype=np.float32)
    target_hidden = np.asarray(target_hidden, dtype=np.float32)
    Wq = np.asarray(Wq, dtype=np.float32)
    Wk = np.asarray(Wk, dtype=np.float32)
    Wv = np.asarray(Wv, dtype=np.float32)
    Wo = np.asarray(Wo, dtype=np.float32)
    q_scale = np.asarray(q_scale, dtype=np.float32)
    k_scale = np.asarray(k_scale, dtype=np.float32)

    x_all = np.concatenate([target_hidden, x_noise], axis=0)       # (S, D)
    xT = np.ascontiguousarray(x_all.T.astype(ml_dtypes.bfloat16))  # (D, S)
    pos_all = np.concatenate(
        [np.asarray(ctx_positions), np.asarray(noise_positions)]
    ).astype(np.float32)
    half = H // 2
    inv_freq = (ROPE_THETA ** (-np.arange(half, dtype=np.float32) * 2.0 / H)
                ).astype(np.float32)
    ang = pos_all[:, None] * inv_freq[None, :]                     # (S, 64)
    sinb = np.ascontiguousarray(np.sin(ang).astype(ml_dtypes.bfloat16))
    cosb = np.ascontiguousarray(np.cos(ang).astype(ml_dtypes.bfloat16))
    qscaleb = np.ascontiguousarray(np.broadcast_to(q_scale, (P, H)))
    kscaleb = np.ascontiguousarray(np.broadcast_to(k_scale, (P, H)))

    in_maps = []
    for c in range(N_CORES):
        wqkv = np.ascontiguousarray(np.concatenate(
            [Wk[:, c, :], Wv[:, c, :],
             Wq[:, 2 * c, :], Wq[:, 2 * c + 1, :]],
            axis=1).astype(ml_dtypes.bfloat16))                     # (D, 512)
        wo_c = np.ascontiguousarray(
            Wo[c * HEADS_PER_CORE:(c + 1) * HEADS_PER_CORE]
            .astype(ml_dtypes.bfloat16))                            # (2,128,D)
        in_maps.append({
            "xT": xT, "wqkv": wqkv, "wo": wo_c,
            "sinb": sinb, "cosb": cosb,
            "qscaleb": qscaleb, "kscaleb": kscaleb,
        })
    return in_maps


def kernel(**inputs):
    in_maps = prepare_in_maps(**inputs)
    nc, out_name = _get_program()
    res = run_bass_kernel_spmd(nc, in_maps, core_ids=list(range(N_CORES)))
    acc = np.zeros((T_NOISE, D), dtype=np.float32)
    for r in res.results:
        acc += r[out_name]
    return acc


def run_traced(inputs, **kw):
    """Run once with NTFF tracing; returns BassKernelResults (exec_time_ns)."""
    in_maps = prepare_in_maps(**inputs)
    nc, out_name = _get_program()
    return run_bass_kernel_spmd(nc, in_maps, core_ids=list(range(N_CORES)),
                                trace=True, **kw)


# revision 13
# speedup vs baseline: 2.3655x; 1.1411x over previous
"""DFlashAttention Trainium2 kernel (8-core tensor-parallel over attention heads).

Shapes (hardcoded): D=2048, N=16 q-heads, K=8 kv-heads, H=128,
T_NOISE=2048 (query tokens), T_CTX=4096, S=6144 (kv tokens).

Sharding: core c owns q-heads {2c, 2c+1} and kv-head c (GQA groups=2).
Each core computes a partial (T, D) output (its 2 heads' slice of the
o-projection contraction); the host sums the 8 partials (TP unshard).

v4 layout strategy per core (all matmul operands bf16, PSUM fp32):
  - x_all^T [D, S] fed bf16; ONE big DMA per 512-token chunk, first two
    chunks prefetched before every other constant load so the PE starts
    within ~10 us.
  - fused QKV projection: one weight tile [wk|wv|wq0|wq1] per d-tile;
    noise-token chunks project q in the same matmul group as k/v
    (512-wide moving operand), so x is loaded exactly once. Chunks are
    emitted noise/ctx interleaved and each chunk's transposes are
    deferred behind the next chunk's matmuls (in-order PE queue hazard).
  - RMSNorm: per-chunk batched stats (Squares w/ accum, one Sqrt, one
    reciprocal); the normalize copy runs on ScalarE (Copy w/ per-token
    scale); the rms scale vector is folded into HOST-precomputed
    rope tables (cA=cos*s1, sA=sin*s2, cB=cos*s2, sB=sin*s1), and the
    rope multiplies run 4-tiles-at-a-time on strided APs.
  - attention in [s, t] orientation, software-pipelined: scores^T for
    pair i+1 are issued before A@V of pair i so the PE streams through
    exp latency. exp on ACT straight to bf16; no max subtraction.
    Row-sums: exp tiles are quad-summed on the (idle) vector engine and
    a single ones-matmul per 4 s-tiles accumulates the denominators
    (third fewer TensorE instructions in the attention inner loop).
  - softmax normalization applied at the oT copy: row-sums -> SBUF ->
    GpSimd partition_broadcast -> reciprocal -> one tensor_mul.
    Phase D is a pure 2-head accumulating o-projection, PSUM evacuated
    alternately by ScalarE/VectorE, one 1 MB store per t-tile.
"""

import sys

for _p in ("/opt/trn_rl_repo", "/root/.axon_site/_ro/trn_rl_repo"):
    if _p not in sys.path:
        sys.path.append(_p)

import math
import numpy as np
import ml_dtypes

import concourse.bass as bass
import concourse.tile as tile
from concourse import bacc
from concourse import mybir
from concourse.bass_utils import run_bass_kernel_spmd
from concourse.masks import make_identity

D = 2048
N_HEADS = 16
K_HEADS = 8
H = 128
T_NOISE = 2048
T_CTX = 4096
S_ALL = T_CTX + T_NOISE          # 6144
EPS = 1e-6
ROPE_THETA = 1e6
N_CORES = 8
HEADS_PER_CORE = N_HEADS // N_CORES   # 2

P = 128                       # partition dim
S_TILES = S_ALL // P          # 48
T_TILES = T_NOISE // P        # 16
NOISE_TILE0 = T_CTX // P      # 32  (noise tokens are s-tiles 32..47)
D_TILES = D // P              # 16
FREE = 512                    # moving free-dim chunk
T_CHUNKS = T_NOISE // FREE    # 4
S_CHUNKS = S_ALL // FREE      # 12
D_CHUNKS = D // FREE          # 4
NOISE_CHUNK0 = T_CTX // FREE  # 8   (chunks 8..11 are noise tokens)
HALF = H // 2

F32 = mybir.dt.float32
BF16 = mybir.dt.bfloat16
MM_DT = BF16                  # dtype for all matmul operands

INV_SQRT_H = 1.0 / math.sqrt(H)
MULT = mybir.AluOpType.mult

_CACHE = {}

# phase-A chunk emission order: noise/ctx interleaved so adjacent chunks
# never need two full noise PSUM slot sets at once, and the PE always has
# a fresh chunk's matmuls to run while the previous chunk's norm drains.
CHUNK_ORDER = [8, 0, 9, 1, 10, 2, 11, 3, 4, 5, 6, 7]


def _build_program(reps=1):
    """Build the single-core SPMD bass program. Returns (nc, out_name)."""
    nc = bacc.Bacc("TRN2", target_bir_lowering=False, debug=False,
                   num_devices=N_CORES)

    xT = nc.dram_tensor("xT", [D, S_ALL], MM_DT, kind="ExternalInput").ap()
    wqkv = nc.dram_tensor("wqkv", [D, 4 * H], MM_DT, kind="ExternalInput").ap()
    wo = nc.dram_tensor("wo", [HEADS_PER_CORE, H, D], MM_DT,
                        kind="ExternalInput").ap()
    # rope tables with the rms-norm scale vectors folded in:
    # [cA, sA, cB, sB] where xr1 = x1*cA - x2*sA ; xr2 = x2*cB + x1*sB
    ktab = nc.dram_tensor("ktab", [4, S_ALL, HALF], MM_DT,
                          kind="ExternalInput").ap()
    qtab = nc.dram_tensor("qtab", [4, T_NOISE, HALF], MM_DT,
                          kind="ExternalInput").ap()
    out = nc.dram_tensor("out", [T_NOISE, D], F32, kind="ExternalOutput").ap()

    with tile.TileContext(nc) as tc:
        for rep in range(reps):
            _emit(nc, tc, xT, wqkv, wo, ktab, qtab, out, pfx=f"r{rep}_")
    nc.compile()
    return nc, "out"


def _emit(nc, tc, xT, wqkv, wo, ktab, qtab, out, pfx=""):
    import contextlib
    ctx = contextlib.ExitStack()
    xTr = xT.rearrange("(d p) s -> p d s", p=P)
    with ctx:
        const = ctx.enter_context(tc.tile_pool(name=pfx + "const", bufs=1))
        persist = ctx.enter_context(tc.tile_pool(name=pfx + "persist", bufs=1))
        xp = ctx.enter_context(tc.tile_pool(name=pfx + "pa_x", bufs=3))

        # ---- weights + first x chunks first: PE starts ASAP ----
        wqkv_sb = const.tile([P, D_TILES * 4 * H], MM_DT, tag="wqkv")
        nc.sync.dma_start(wqkv_sb[:],
                          wqkv.rearrange("(d p) c -> p d c", p=P))
        prefetched = {}
        for sc in CHUNK_ORDER[:2]:
            xt = xp.tile([P, D_TILES * FREE], MM_DT, tag="xc", name="xchunk")
            nc.sync.dma_start(xt[:], xTr[:, :, sc * FREE:(sc + 1) * FREE])
            prefetched[sc] = xt

        # ---- remaining constants ----
        ident = const.tile([P, P], MM_DT, tag="ident")
        make_identity(nc, ident[:])
        ones = const.tile([P, 1], MM_DT, tag="ones")
        nc.vector.memset(ones[:], 1.0)
        eps_col = const.tile([P, 1], F32, tag="eps")
        nc.vector.memset(eps_col[:], EPS)
        ktab_sb = [persist.tile([P, S_TILES * HALF], MM_DT, tag=f"ktab{i}",
                                name=f"ktab{i}") for i in range(4)]
        qtab_sb = [persist.tile([P, T_TILES * HALF], MM_DT, tag=f"qtab{i}",
                                name=f"qtab{i}") for i in range(4)]
        for i in range(4):
            nc.sync.dma_start(ktab_sb[i][:],
                              ktab[i].rearrange("(t p) f -> p t f", p=P))
            nc.sync.dma_start(qtab_sb[i][:],
                              qtab[i].rearrange("(t p) f -> p t f", p=P))
        wo_sb = [const.tile([P, D], MM_DT, tag=f"wo{h}", name=f"wos{h}")
                 for h in range(HEADS_PER_CORE)]
        for h in range(HEADS_PER_CORE):
            nc.sync.dma_start(wo_sb[h][:], wo[h])

        # ---- persistent activations ----
        kT_sb = persist.tile([P, S_ALL], MM_DT, tag="kT")
        v_sb = persist.tile([P, S_ALL], MM_DT, tag="v")     # [s-tile, h] blocks
        qT_sb = persist.tile([P, HEADS_PER_CORE * T_NOISE], MM_DT, tag="qT")
        oT_sb = persist.tile([P, HEADS_PER_CORE * T_NOISE], MM_DT, tag="oT")

        # ---- Phase A: fused QKV projection + norm/rope for all chunks ----
        with tc.tile_pool(name=pfx + "pa_ps", bufs=4, space="PSUM") as pa, \
             tc.tile_pool(name=pfx + "pa_ps2", bufs=2, space="PSUM") as pa2, \
             tc.tile_pool(name=pfx + "pa_pt", bufs=2, space="PSUM") as pst, \
             tc.tile_pool(name=pfx + "pa_w", bufs=4) as work, \
             tc.tile_pool(name=pfx + "pa_xr", bufs=6) as xrp:
            pending = []   # deferred transposes: (xr_row, [(j, dst), ...])

            def flush_pending():
                for xr_row, dsts in pending:
                    for j, dst in dsts:
                        pt = pst.tile([P, P], MM_DT, tag="pt")
                        nc.tensor.transpose(
                            pt[:], xr_row[:, j * P:(j + 1) * P], ident[:])
                        nc.vector.tensor_copy(dst, pt[:])
                pending.clear()

            def norm_rope_row(slots, off, tabs, tbase, rinv, rcol0, dsts):
                """Normalize+rope 4 token tiles (one 'row') at once.
                slots: 4 PSUM tiles; off: column offset of this row's H block;
                tabs: 4 table tiles; tbase: first table tile index;
                rinv: [P,12] stats tile, rcol0: first rinv column;
                dsts: 4 destination [P,128] SBUF slices (transposed)."""
                xn = xrp.tile([P, 4 * H], MM_DT, tag="xn")
                for j in range(4):
                    nc.scalar.activation(
                        xn[:, j * H:(j + 1) * H], slots[j][:, off:off + H],
                        mybir.ActivationFunctionType.Copy,
                        scale=rinv[:, rcol0 + j:rcol0 + j + 1])
                xnv = xn[:].rearrange("p (j h) -> p j h", j=4)
                x1 = xnv[:, :, 0:HALF]
                x2 = xnv[:, :, HALF:H]
                tsl = slice(tbase * HALF, (tbase + 4) * HALF)
                cA = tabs[0][:, tsl].rearrange("p (j f) -> p j f", j=4)
                sA = tabs[1][:, tsl].rearrange("p (j f) -> p j f", j=4)
                cB = tabs[2][:, tsl].rearrange("p (j f) -> p j f", j=4)
                sB = tabs[3][:, tsl].rearrange("p (j f) -> p j f", j=4)
                t1 = xrp.tile([P, 4 * HALF], MM_DT, tag="t1")
                t2 = xrp.tile([P, 4 * HALF], MM_DT, tag="t2")
                t1v = t1[:].rearrange("p (j f) -> p j f", j=4)
                t2v = t2[:].rearrange("p (j f) -> p j f", j=4)
                xr = xrp.tile([P, 4 * H], MM_DT, tag="xr")
                xrv = xr[:].rearrange("p (j h) -> p j h", j=4)
                nc.vector.tensor_mul(t1v, x1, cA)
                nc.vector.tensor_mul(t2v, x2, sA)
                nc.vector.tensor_sub(xrv[:, :, 0:HALF], t1v, t2v)
                nc.vector.tensor_mul(t1v, x2, cB)
                nc.vector.tensor_mul(t2v, x1, sB)
                nc.vector.tensor_add(xrv[:, :, HALF:H], t1v, t2v)
                pending.append((xr, dsts))

            for sc in CHUNK_ORDER:
                noise = sc >= NOISE_CHUNK0
                w_w = 4 * H if noise else 2 * H
                xt = prefetched.pop(sc, None)
                if xt is None:
                    xt = xp.tile([P, D_TILES * FREE], MM_DT, tag="xc",
                                 name="xchunk")
                    nc.sync.dma_start(
                        xt[:], xTr[:, :, sc * FREE:(sc + 1) * FREE])
                if noise:
                    slots = [pa.tile([P, w_w], F32, tag="pjn", name=f"pjn{j}")
                             for j in range(4)]
                else:
                    # pack two 256-wide ctx outputs per PSUM bank
                    banks = [pa2.tile([P, 2 * w_w], F32, tag="pjc",
                                      name=f"pjc{b}") for b in range(2)]
                    slots = [banks[j // 2][:, (j % 2) * w_w:(j % 2 + 1) * w_w]
                             for j in range(4)]
                # j outer, d inner: each j's accumulation group runs start..
                # stop without another group's start= clearing its bank's
                # has_written bits (two ctx groups share one PSUM bank).
                for j in range(4):
                    for d in range(D_TILES):
                        nc.tensor.matmul(
                            slots[j][:],
                            xt[:, d * FREE + j * P:d * FREE + (j + 1) * P],
                            wqkv_sb[:, d * 4 * H:d * 4 * H + w_w],
                            start=(d == 0), stop=(d == D_TILES - 1))
                flush_pending()   # prev chunk's transposes, inputs now ready
                # batched RMS stats: k per j (cols 0..3), q0 (4..7), q1 (8..11)
                ncols = 12 if noise else 4
                ssq = work.tile([P, 12], F32, tag="ssq")
                rms = work.tile([P, 12], F32, tag="rms")
                rinv = work.tile([P, 12], F32, tag="rinv")
                sq = [work.tile([P, H], F32, tag="sq", name=f"sq{j}")
                      for j in range(4)]
                for j in range(4):
                    nc.scalar.activation(
                        sq[j][:], slots[j][:, 0:H],
                        mybir.ActivationFunctionType.Square,
                        accum_out=ssq[:, j:j + 1])
                if noise:
                    for hh in range(HEADS_PER_CORE):
                        for j in range(4):
                            nc.scalar.activation(
                                sq[j][:], slots[j][:, (2 + hh) * H:(3 + hh) * H],
                                mybir.ActivationFunctionType.Square,
                                accum_out=ssq[:, 4 + 4 * hh + j:5 + 4 * hh + j])
                nc.scalar.activation(rms[:, 0:ncols], ssq[:, 0:ncols],
                                     mybir.ActivationFunctionType.Sqrt,
                                     bias=eps_col[:], scale=1.0 / H)
                nc.vector.reciprocal(rinv[:, 0:ncols], rms[:, 0:ncols])
                for j in range(4):
                    si = sc * 4 + j
                    nc.vector.tensor_copy(v_sb[:, si * P:(si + 1) * P],
                                          slots[j][:, H:2 * H])
                norm_rope_row(
                    slots, 0, ktab_sb, sc * 4, rinv, 0,
                    [(j, kT_sb[:, (sc * 4 + j) * P:(sc * 4 + j + 1) * P])
                     for j in range(4)])
                if noise:
                    ti0 = (sc - NOISE_CHUNK0) * 4
                    for hh in range(HEADS_PER_CORE):
                        norm_rope_row(
                            slots, (2 + hh) * H, qtab_sb, ti0, rinv,
                            4 + 4 * hh,
                            [(j, qT_sb[:, hh * T_NOISE + (ti0 + j) * P:
                                       hh * T_NOISE + (ti0 + j + 1) * P])
                             for j in range(4)])
            flush_pending()

        # ---- Phase C: attention (software-pipelined, quad row-sums) ----
        PAIR = 2 * FREE   # exp processes two score banks at once
        NP_ = S_TILES // 2
        NQ_ = S_TILES // 4
        with tc.tile_pool(name=pfx + "pc_sc", bufs=2, space="PSUM") as psc, \
             tc.tile_pool(name=pfx + "pc_av", bufs=2, space="PSUM") as pav, \
             tc.tile_pool(name=pfx + "pc_r", bufs=2, space="PSUM") as pr, \
             tc.tile_pool(name=pfx + "pc_exp", bufs=4) as pexp, \
             tc.tile_pool(name=pfx + "pc_w", bufs=2) as cwork:
            for hh in range(HEADS_PER_CORE):
                for tch in range(T_CHUNKS):
                    qslice = qT_sb[:, hh * T_NOISE + tch * FREE:
                                   hh * T_NOISE + (tch + 1) * FREE]
                    av = pav.tile([P, FREE], F32, tag="av")
                    rr = pr.tile([1, FREE], F32, tag="rr")
                    qpend = []
                    qstate = [0]

                    def scores_exp(sp):
                        sc_ps = psc.tile([P, PAIR], F32, tag="sc")
                        ex = pexp.tile([P, PAIR], MM_DT, tag="ex")
                        for u in range(2):
                            si = sp * 2 + u
                            nc.tensor.matmul(
                                sc_ps[:, u * FREE:(u + 1) * FREE],
                                kT_sb[:, si * P:(si + 1) * P], qslice,
                                start=True, stop=True)
                        nc.scalar.activation(ex[:], sc_ps[:],
                                             mybir.ActivationFunctionType.Exp,
                                             scale=INV_SQRT_H)
                        return ex

                    def av_mm(sp, ex):
                        for u in range(2):
                            si = sp * 2 + u
                            nc.tensor.matmul(
                                av[:], v_sb[:, si * P:(si + 1) * P],
                                ex[:, u * FREE:(u + 1) * FREE],
                                start=(si == 0), stop=(si == S_TILES - 1))
                        qpend.append(ex)
                        if len(qpend) == 2:
                            exA, exB = qpend
                            t0 = cwork.tile([P, FREE], MM_DT, tag="q0")
                            t1 = cwork.tile([P, FREE], MM_DT, tag="q1")
                            qs = cwork.tile([P, FREE], MM_DT, tag="qs")
                            nc.vector.tensor_add(t0[:], exA[:, 0:FREE],
                                                 exA[:, FREE:PAIR])
                            nc.vector.tensor_add(t1[:], exB[:, 0:FREE],
                                                 exB[:, FREE:PAIR])
                            nc.vector.tensor_add(qs[:], t0[:], t1[:])
                            qi = qstate[0]
                            nc.tensor.matmul(rr[:], ones[:], qs[:],
                                             start=(qi == 0),
                                             stop=(qi == NQ_ - 1))
                            qstate[0] = qi + 1
                            qpend.clear()

                    ex_prev = scores_exp(0)
                    for sp in range(1, NP_):
                        ex_cur = scores_exp(sp)
                        av_mm(sp - 1, ex_prev)
                        ex_prev = ex_cur
                    av_mm(NP_ - 1, ex_prev)

                    # normalize: oT = av / rowsum (broadcast recip over h)
                    cbase = hh * T_NOISE + tch * FREE
                    rrow = cwork.tile([1, FREE], F32, tag="rrow")
                    nc.vector.tensor_copy(rrow[:], rr[:])
                    rb = cwork.tile([P, FREE], F32, tag="rb")
                    nc.gpsimd.partition_broadcast(rb[:], rrow[:])
                    nc.vector.reciprocal(rb[:], rb[:])
                    nc.vector.tensor_mul(oT_sb[:, cbase:cbase + FREE],
                                         av[:], rb[:])

        # ---- Phase D: o-projection (2-head accumulate) + store ----
        with tc.tile_pool(name=pfx + "pd_ps", bufs=3, space="PSUM") as pso, \
             tc.tile_pool(name=pfx + "pd_o", bufs=3) as ostage:
            for ti in range(T_TILES):
                ot = ostage.tile([P, D], F32, tag="ot", name="otile")
                for dc in range(D_CHUNKS):
                    po = pso.tile([P, FREE], F32, tag="po")
                    for h in range(HEADS_PER_CORE):
                        nc.tensor.matmul(
                            po[:],
                            oT_sb[:, h * T_NOISE + ti * P:
                                  h * T_NOISE + (ti + 1) * P],
                            wo_sb[h][:, dc * FREE:(dc + 1) * FREE],
                            start=(h == 0), stop=(h == HEADS_PER_CORE - 1))
                    if dc % 2 == 0:
                        nc.vector.tensor_copy(
                            ot[:, dc * FREE:(dc + 1) * FREE], po[:])
                    else:
                        nc.scalar.activation(
                            ot[:, dc * FREE:(dc + 1) * FREE], po[:],
                            mybir.ActivationFunctionType.Copy)
                nc.sync.dma_start(out[ti * P:(ti + 1) * P, :], ot[:])


def _get_program(reps=1):
    key = f"prog{reps}"
    if key not in _CACHE:
        _CACHE[key] = _build_program(reps)
    return _CACHE[key]


def prepare_in_maps(x_noise, target_hidden, Wq, Wk, Wv, Wo, q_scale, k_scale,
                    noise_positions, ctx_positions):
    x_noise = np.asarray(x_noise, dtype=np.float32)
    target_hidden = np.asarray(target_hidden, dtype=np.float32)
    Wq = np.asarray(Wq, dtype=np.float32)
    Wk = np.asarray(Wk, dtype=np.float32)
    Wv = np.asarray(Wv, dtype=np.float32)
    Wo = np.asarray(Wo, dtype=np.float32)
    q_scale = np.asarray(q_scale, dtype=np.float32)
    k_scale = np.asarray(k_scale, dtype=np.float32)

    x_all = np.concatenate([target_hidden, x_noise], axis=0)       # (S, D)
    xT = np.ascontiguousarray(x_all.T.astype(ml_dtypes.bfloat16))  # (D, S)
    pos_all = np.concatenate(
        [np.asarray(ctx_positions), np.asarray(noise_positions)]
    ).astype(np.float32)
    inv_freq = (ROPE_THETA ** (-np.arange(HALF, dtype=np.float32) * 2.0 / H)
                ).astype(np.float32)

    def rope_tabs(pos, scale):
        ang = pos[:, None] * inv_freq[None, :]
        co, sn = np.cos(ang), np.sin(ang)
        s1, s2 = scale[0:HALF], scale[HALF:H]
        return np.ascontiguousarray(np.stack(
            [co * s1, sn * s2, co * s2, sn * s1]
        ).astype(ml_dtypes.bfloat16))

    ktab = rope_tabs(pos_all, k_scale)                 # (4, S, 64)
    qtab = rope_tabs(pos_all[T_CTX:], q_scale)         # (4, T, 64)

    in_maps = []
    for c in range(N_CORES):
        wqkv = np.ascontiguousarray(np.concatenate(
            [Wk[:, c, :], Wv[:, c, :],
             Wq[:, 2 * c, :], Wq[:, 2 * c + 1, :]],
            axis=1).astype(ml_dtypes.bfloat16))                     # (D, 512)
        wo_c = np.ascontiguousarray(
            Wo[c * HEADS_PER_CORE:(c + 1) * HEADS_PER_CORE]
            .astype(ml_dtypes.bfloat16))                            # (2,128,D)
        in_maps.append({
            "xT": xT, "wqkv": wqkv, "wo": wo_c,
            "ktab": ktab, "qtab": qtab,
        })
    return in_maps


def kernel(**inputs):
    in_maps = prepare_in_maps(**inputs)
    nc, out_name = _get_program()
    res = run_bass_kernel_spmd(nc, in_maps, core_ids=list(range(N_CORES)))
    acc = np.zeros((T_NOISE, D), dtype=np.float32)
    for r in res.results:
        acc += r[out_name]
    return acc


def run_traced(inputs, **kw):
    """Run once with NTFF tracing; returns BassKernelResults (exec_time_ns)."""
    in_maps = prepare_in_maps(**inputs)
    nc, out_name = _get_program()
    return run_bass_kernel_spmd(nc, in_maps, core_ids=list(range(N_CORES)),
                                trace=True, **kw)
